# revision 1
# baseline (speedup 1.0000x reference)
"""DeepSeekV2-style MLA decode attention (MQA, B=128 decode tokens) on 8 trn2 NeuronCores.

Strategy (all shapes hardcoded from the problem spec):
  - Heads tensor-parallel for the projections: core c owns 16 heads (Wq cols /
    Wo rows sharded); Wkv replicated (tiny).
  - Batch-parallel for attention: core c owns 16 of the 128 decode batches;
    k/v caches are sharded by batch, truncated to seq_len, and staged by the
    host (kT pre-transposed) so the device streams only valid positions.
  - Two on-device AllToAlls reshard q (head-shard -> batch-shard) and attn
    output (batch-shard -> head-shard).  Final Wo partials are summed on host.
  - New-token kv (h @ Wkv) is computed on-device; the new token occupies
    position 0 of each staged cache slot, so its patch location is static.
  - Everything enters the PE as bf16 (host-cast); accumulation is f32.

The device program is specialized on seq_lens (static loop bounds); kernel()
re-builds/compiles whenever the derived schedule changes and memoizes it.
"""

import os
import numpy as np
import ml_dtypes

import concourse.bass as bass
import concourse.mybir as mybir
import concourse.tile as tile
from concourse import bacc
from concourse.bass_utils import run_bass_kernel_spmd

BF16 = ml_dtypes.bfloat16
P = 128
B, MAX_S, HID = 128, 4096, 5120
H, D, D_ROPE, D_V = 128, 128, 64, 128
NC, HPC, BPC = 8, 16, 16           # cores, heads/core, slots(batches)/core
QCOLS = HPC * D + 256              # 2304: per-core q cols + kv cols
SCALE = float(D) ** -0.5
PROJ_CHUNKS = [(0, 512), (512, 512), (1024, 512), (1536, 512), (2048, 256)]
WO_PASSES = [(0, 1024), (1024, 1024), (2048, 1024), (3072, 1024), (4096, 1024)]

dtb = mybir.dt.bfloat16
dtf = mybir.dt.float32

_PROGRAM_CACHE: dict = {}
LAST_RESULTS = None


def _install_ntff_hook():
    """The agent image's ``antenv`` lacks ``axon_hooks``; recreate it so
    run_bass_kernel_spmd(trace=True) can capture NTFF profiles via the
    libaxon ctypes path. Best-effort — failure just means no trace."""
    import sys
    import types
    try:
        import antenv.axon_hooks  # noqa: F401
        return
    except ImportError:
        pass
    try:
        from trn_agent_boot.trn_boot import _ntff_profile_via_ctypes
        hook = _ntff_profile_via_ctypes("/opt/axon/libaxon_pjrt.so")
        mod = types.ModuleType("antenv.axon_hooks")
        mod._hook = hook
        mod.set_axon_ntff_profile_hook = lambda h: setattr(mod, "_hook", h)
        mod.get_axon_ntff_profile_hook = lambda: mod._hook
        sys.modules["antenv.axon_hooks"] = mod
        import antenv
        antenv.axon_hooks = mod
    except Exception:
        pass


def _build_program(T, toff, NTT):
    """Build + compile the per-core bass program. T/toff: per-slot tile counts
    and tile offsets (static schedule, same on every core). NTT = sum(T)."""
    nc = bacc.Bacc("TRN2", target_bir_lowering=False, debug=False, num_devices=NC)

    ht_d = nc.dram_tensor("ht", [P, HID], dtb, kind="ExternalInput")
    wq_d = nc.dram_tensor("wq", [HID, QCOLS], dtb, kind="ExternalInput")
    kt_d = nc.dram_tensor("kt", [P, NTT * P], dtb, kind="ExternalInput")
    vv_d = nc.dram_tensor("vv", [P, NTT * P], dtb, kind="ExternalInput")
    bias_d = nc.dram_tensor("bias", [P, NTT], dtf, kind="ExternalInput")
    wo_d = nc.dram_tensor("wo", [HPC * D_V, HID], dtb, kind="ExternalInput")
    idn_d = nc.dram_tensor("idn", [P, P], dtb, kind="ExternalInput")
    one_d = nc.dram_tensor("one", [P, 1], dtb, kind="ExternalInput")
    out_d = nc.dram_tensor("outp", [P, HID], dtf, kind="ExternalOutput")
    debug = os.environ.get("BASS_KERNEL_DEBUG", "0") == "1"
    if debug:
        dbg_qin = nc.dram_tensor("dbg_qin", [P, QCOLS], dtb, kind="ExternalOutput")
        dbg_qout = nc.dram_tensor("dbg_qout", [P, QCOLS], dtb, kind="ExternalOutput")
        dbg_at = nc.dram_tensor("dbg_at", [P, BPC * D_V], dtb, kind="ExternalOutput")
        dbg_ao = nc.dram_tensor("dbg_ao", [P, BPC * D_V], dtb, kind="ExternalOutput")
        dbg_sum = nc.dram_tensor("dbg_sum", [P, BPC], dtf, kind="ExternalOutput")
        dbg_et = nc.dram_tensor("dbg_et", [P, NTT * P], dtb, kind="ExternalOutput")
        dbg_qjt = nc.dram_tensor("dbg_qjt", [P, BPC * P], dtb, kind="ExternalOutput")
        dbg_kt = nc.dram_tensor("dbg_kt", [P, BPC * P], dtb, kind="ExternalOutput")

    rg = [list(range(NC))]
    Exp = mybir.ActivationFunctionType.Exp
    Copy = mybir.ActivationFunctionType.Copy

    with tile.TileContext(nc) as tc:
        with (
            tc.tile_pool(name="cpool", bufs=1) as cpool,
            tc.tile_pool(name="wpool", bufs=3) as wpool,
            tc.tile_pool(name="kvpool", bufs=2) as kvpool,
            tc.tile_pool(name="spool", bufs=3) as spool,
            tc.tile_pool(name="epool", bufs=6) as epool,
            tc.tile_pool(name="lhpool", bufs=HPC) as lhpool,
            tc.tile_pool(name="opool", bufs=2) as opool,
            tc.tile_pool(name="dram", bufs=1, space="DRAM") as dram,
        ):
            # ---- constants / global loads ----
            idn = cpool.tile([P, P], dtb)
            nc.sync.dma_start(idn[:], idn_d.ap())
            ones = cpool.tile([P, 1], dtb)
            nc.sync.dma_start(ones[:], one_d.ap())
            hts = cpool.tile([P, HID], dtb)
            nc.sync.dma_start(hts[:], ht_d.ap())
            bias_sb = cpool.tile([P, NTT], dtf)
            nc.sync.dma_start(bias_sb[:], bias_d.ap())

            qcc_in = dram.tile([P, QCOLS], dtb)
            qcc_out = dram.tile([P, QCOLS], dtb)
            acc_in = dram.tile([P, BPC * D_V], dtb)
            acc_out = dram.tile([P, BPC * D_V], dtb)

            # ---- phase A: q/kv projection (this core's 16 heads, all 128 batches) ----
            with tc.tile_pool(name="pjp", bufs=1, space="PSUM") as pjp:
                q_ps = pjp.tile([P, QCOLS], dtf)
                for k in range(HID // P):
                    wt = wpool.tile([P, QCOLS], dtb, tag="wt")
                    nc.sync.dma_start(wt[:], wq_d.ap()[k * P:(k + 1) * P, :])
                    for (c0, cw) in PROJ_CHUNKS:
                        nc.tensor.matmul(
                            q_ps[:, c0:c0 + cw],
                            lhsT=hts[:, k * P:(k + 1) * P],
                            rhs=wt[:, c0:c0 + cw],
                            start=(k == 0),
                            stop=(k == HID // P - 1),
                        )
                qc_sb = cpool.tile([P, QCOLS], dtb)
                nc.vector.tensor_copy(qc_sb[:], q_ps[:])

            nc.sync.dma_start(qcc_in[:], qc_sb[:])
            nc.gpsimd.collective_compute(
                "AllToAll", mybir.AluOpType.bypass, replica_groups=rg,
                ins=[qcc_in.opt()], outs=[qcc_out.opt()],
            )
            if debug:
                nc.sync.dma_start(dbg_qin.ap(), qc_sb[:])
                nc.sync.dma_start(dbg_qout.ap(), qcc_out[:])

            # ---- phase B: attention over this core's 16 batches ----
            attn_cc = cpool.tile([P, BPC * D_V], dtb)
            with (
                tc.tile_pool(name="tps", bufs=2, space="PSUM") as tps,
                tc.tile_pool(name="scps", bufs=2, space="PSUM") as scps,
                tc.tile_pool(name="aps", bufs=2, space="PSUM") as aps,
                tc.tile_pool(name="sps", bufs=2, space="PSUM") as sps,
            ):
                # new-token k/v from the AllToAll'd kv columns (identical from
                # every source core; read source block 0)
                knew = cpool.tile([BPC, P], dtb)
                nc.sync.dma_start(knew[:], qcc_out[0:BPC, HPC * D:HPC * D + P])
                knewT_ps = tps.tile([P, BPC], dtb, tag="tps")
                nc.tensor.transpose(knewT_ps[:], knew[:], idn[0:BPC, 0:BPC])
                knewT = cpool.tile([P, BPC], dtb)
                nc.vector.tensor_copy(knewT[:], knewT_ps[:])
                vnew = cpool.tile([1, BPC * P], dtb)
                nc.sync.dma_start(
                    vnew.rearrange("one (j d) -> one j d", d=P),
                    qcc_out.rearrange("(c j) f -> c j f", j=BPC)[0:1, :, HPC * D + P:QCOLS],
                )

                for j in range(BPC):
                    Tj = int(T[j])
                    # assemble q_j [128 h, 128 d] from gathered rows, then transpose
                    qj = spool.tile([P, P], dtb, tag="qj")
                    src = (
                        qcc_out.rearrange("(c j) f -> c j f", j=BPC)[:, j:j + 1, 0:HPC * D]
                        .squeeze(1)
                        .rearrange("c (hl d) -> c hl d", d=P)
                    )
                    nc.sync.dma_start(qj[:], src)
                    qjT_ps = tps.tile([P, P], dtb, tag="tps")
                    nc.tensor.transpose(qjT_ps[:], qj[:], idn[:])
                    qjT = spool.tile([P, P], dtb, tag="qjT")
                    nc.vector.tensor_copy(qjT[:], qjT_ps[:])
                    if debug:
                        nc.sync.dma_start(dbg_qjt.ap()[:, j * P:(j + 1) * P], qjT[:])

                    # staged caches for this slot
                    kt_sb = kvpool.tile([P, Tj * P], dtb, tag="kt")
                    nc.sync.dma_start(kt_sb[:], kt_d.ap()[:, toff[j] * P:(toff[j] + Tj) * P])
                    nc.vector.tensor_copy(kt_sb[:, 0:1], knewT[:, j:j + 1])
                    if debug:
                        nc.sync.dma_start(dbg_kt.ap()[:, j * P:(j + 1) * P], kt_sb[:, 0:P])
                    vv_sb = kvpool.tile([P, Tj * P], dtb, tag="vv")
                    nc.sync.dma_start(vv_sb[:], vv_d.ap()[:, toff[j] * P:(toff[j] + Tj) * P])
                    # new token sits at slot position 0 = partition 0 of v-tile 0
                    nc.vector.tensor_copy(vv_sb[0:1, 0:P], vnew[0:1, j * P:(j + 1) * P])

                    attn_ps = aps.tile([P, D_V], dtf, tag="attn")
                    sum_ps = sps.tile([P, 1], dtf, tag="sum")
                    for t in range(Tj):
                        sc_ps = scps.tile([P, P], dtf, tag="sc")
                        nc.tensor.matmul(
                            sc_ps[:], lhsT=kt_sb[:, t * P:(t + 1) * P], rhs=qjT[:],
                            start=True, stop=True,
                        )
                        et = epool.tile([P, P], dtb, tag="et")
                        nc.scalar.activation(
                            et[:], sc_ps[:], Exp,
                            bias=bias_sb[:, toff[j] + t:toff[j] + t + 1], scale=SCALE,
                        )
                        if debug:
                            nc.sync.dma_start(
                                dbg_et.ap()[:, (toff[j] + t) * P:(toff[j] + t + 1) * P],
                                et[:],
                            )
                        nc.tensor.matmul(
                            attn_ps[:], lhsT=et[:], rhs=vv_sb[:, t * P:(t + 1) * P],
                            start=(t == 0), stop=(t == Tj - 1), skip_group_check=True,
                        )
                        nc.tensor.matmul(
                            sum_ps[:], lhsT=et[:], rhs=ones[:],
                            start=(t == 0), stop=(t == Tj - 1), skip_group_check=True,
                        )
                    sum_sb = spool.tile([P, 1], dtf, tag="sumsb")
                    nc.vector.tensor_copy(sum_sb[:], sum_ps[:])
                    if debug:
                        nc.sync.dma_start(dbg_sum.ap()[:, j:j + 1], sum_sb[:])
                    recip = spool.tile([P, 1], dtf, tag="recip")
                    nc.vector.reciprocal(recip[:], sum_sb[:])
                    nc.scalar.activation(
                        attn_cc[:, j * D_V:(j + 1) * D_V], attn_ps[:], Copy,
                        bias=0.0, scale=recip[:],
                    )

            nc.sync.dma_start(acc_in[:], attn_cc[:])
            nc.gpsimd.collective_compute(
                "AllToAll", mybir.AluOpType.bypass, replica_groups=rg,
                ins=[acc_in.opt()], outs=[acc_out.opt()],
            )
            if debug:
                nc.sync.dma_start(dbg_at.ap(), attn_cc[:])
                nc.sync.dma_start(dbg_ao.ap(), acc_out[:])

            # ---- phase C: Wo partial (this core's 16 heads, all 128 batches) ----
            with (
                tc.tile_pool(name="tps2", bufs=2, space="PSUM") as tps2,
                tc.tile_pool(name="wops", bufs=2, space="PSUM") as wops,
            ):
                lhs_tiles = []
                for hl in range(HPC):
                    bm = spool.tile([P, P], dtb, tag="bm")
                    src = (
                        acc_out.rearrange("(c hl) f -> c hl f", hl=HPC)[:, hl:hl + 1, :]
                        .squeeze(1)
                        .rearrange("c (j d) -> c j d", d=P)
                    )
                    nc.sync.dma_start(bm[:], src)
                    lh_ps = tps2.tile([P, P], dtb, tag="tps2")
                    nc.tensor.transpose(lh_ps[:], bm[:], idn[:])
                    lh = lhpool.tile([P, P], dtb, tag="lh")
                    nc.vector.tensor_copy(lh[:], lh_ps[:])
                    lhs_tiles.append(lh)
                for (n0, nw) in WO_PASSES:
                    wo_ps = wops.tile([P, 1024], dtf, tag="wop")
                    for kt in range(HPC):
                        wt2 = wpool.tile([P, 1024], dtb, tag="wt2")
                        nc.sync.dma_start(wt2[:], wo_d.ap()[kt * P:(kt + 1) * P, n0:n0 + nw])
                        for half in range(2):
                            nc.tensor.matmul(
                                wo_ps[:, half * 512:(half + 1) * 512],
                                lhsT=lhs_tiles[kt][:],
                                rhs=wt2[:, half * 512:(half + 1) * 512],
                                start=(kt == 0), stop=(kt == HPC - 1),
                            )
                    out_sb = opool.tile([P, 1024], dtf, tag="osb")
                    nc.vector.tensor_copy(out_sb[:], wo_ps[:])
                    nc.sync.dma_start(out_d.ap()[:, n0:n0 + nw], out_sb[:])

    nc.compile()
    return nc


def kernel(hidden_states, k_cache, v_cache, Wq, Wkv, Wo, positions, slot_mapping,
           seq_lens):
    global LAST_RESULTS
    h = np.asarray(hidden_states, np.float32)[:, -1, :]        # [B, HID]
    k_cache = np.asarray(k_cache, np.float32)
    v_cache = np.asarray(v_cache, np.float32)
    Wq = np.asarray(Wq, np.float32)
    Wkv = np.asarray(Wkv, np.float32)
    Wo = np.asarray(Wo, np.float32)
    seq = np.asarray(seq_lens).astype(np.int64)

    # ---- schedule: sort batches by length, slot j holds ranks [8j, 8j+8) ----
    order = np.argsort(-seq, kind="stable")
    batch_of = np.empty((NC, BPC), np.int64)
    for j in range(BPC):
        for c in range(NC):
            batch_of[c, j] = order[NC * j + c]
    perm = batch_of.reshape(-1)                                # sigma (c-major)
    L = np.array([int(seq[order[NC * j]]) for j in range(BPC)])
    T = (L + P - 1) // P
    toff = np.concatenate([[0], np.cumsum(T)])[:-1].astype(np.int64)
    NTT = int(T.sum())

    key = (NTT, tuple(int(t) for t in T), os.environ.get("BASS_KERNEL_DEBUG", "0"))
    if key not in _PROGRAM_CACHE:
        _PROGRAM_CACHE.clear()
        _PROGRAM_CACHE[key] = _build_program(T, toff, NTT)
    nc = _PROGRAM_CACHE[key]

    # ---- host staging ----
    # h^T in device layout: [p, k*128 + b] = h_sigma[b, k*128+p]
    h_sigma = h[perm].astype(BF16)                             # [128, HID]
    ht_stage = np.ascontiguousarray(
        h_sigma.reshape(P, HID // P, P).transpose(2, 1, 0).reshape(P, HID)
    )
    idn_np = np.eye(P, dtype=BF16)
    one_np = np.ones((P, 1), BF16)

    in_maps = []
    for c in range(NC):
        cols = [Wq[:, (HPC * c + hl) * (D + D_ROPE):(HPC * c + hl) * (D + D_ROPE) + D]
                for hl in range(HPC)]
        wq_stage = np.concatenate(cols + [Wkv[:, :256]], axis=1).astype(BF16)
        wo_stage = np.ascontiguousarray(
            Wo[HPC * c * D_V:(HPC * c + HPC) * D_V, :]
        ).astype(BF16)

        kt_stage = np.zeros((P, NTT * P), BF16)
        vv_stage = np.zeros((P, NTT * P), BF16)
        bias_stage = np.full((P, NTT), -30000.0, np.float32)
        for j in range(BPC):
            b = int(batch_of[c, j])
            S = int(seq[b])
            o = int(toff[j]) * P
            # position 0 = new token (k patched on device, v handled by rank-1
            # matmul); positions 1..S-1 = cache rows 0..S-2
            kt_stage[:, o + 1:o + S] = k_cache[b, :S - 1, :].T.astype(BF16)
            vblk = np.zeros((int(T[j]) * P, P), BF16)
            vblk[1:S, :] = v_cache[b, :S - 1, :].astype(BF16)
            vv_stage[:, o:o + int(T[j]) * P] = (
                vblk.reshape(int(T[j]), P, P).transpose(1, 0, 2).reshape(P, -1)
            )
            pidx = np.arange(P)
            for t in range(int(T[j])):
                bias_stage[(t * P + pidx) < S, int(toff[j]) + t] = 0.0
        in_maps.append({
            "ht": ht_stage, "wq": wq_stage, "kt": kt_stage, "vv": vv_stage,
            "bias": bias_stage, "wo": wo_stage, "idn": idn_np, "one": one_np,
        })

    trace = os.environ.get("BASS_KERNEL_TRACE", "0") == "1"
    if trace:
        _install_ntff_hook()
    res = run_bass_kernel_spmd(nc, in_maps, core_ids=list(range(NC)), trace=trace)
    LAST_RESULTS = res
    global LAST_INMAPS, LAST_SCHED
    LAST_INMAPS = in_maps
    LAST_SCHED = (order, batch_of, perm, L, T, toff, NTT)

    out_sigma = np.zeros((P, HID), np.float32)
    for c in range(NC):
        out_sigma += res.results[c]["outp"]
    out_full = np.empty((B, HID), np.float32)
    out_full[perm] = out_sigma
    return out_full.reshape(B, 1, HID)



# revision 2
# speedup vs baseline: 1.3271x; 1.3271x over previous
"""DeepSeekV2-style MLA decode attention (MQA, B=128 decode tokens) on 8 trn2 NeuronCores.

Strategy (all shapes hardcoded from the problem spec):
  - Heads tensor-parallel for the projections: core c owns 16 heads (Wq cols /
    Wo rows sharded).  Batch-parallel for attention: core c owns 16 of the 128
    decode batches; k/v caches are sharded by batch, truncated to seq_len, and
    staged by the host (kT pre-transposed, v carries a ones-column so the
    softmax denominator rides the attention matmul).
  - New-token k/v (h @ Wkv, 0.3% of FLOPs) is computed on host and baked into
    the staged caches at position 0.
  - Two on-device AllToAlls reshard q (head-shard -> batch-shard, split in two
    64-row halves so attention starts during the second transfer) and attn
    output (batch-shard -> head-shard).  Final Wo partials are summed on host.
  - Masking: padded positions have k=0 / v=0 so exp(0)=1 contributes exactly
    n_pad to the denominator, which the host-staged pad count subtracts.
  - Weights/caches enter the PE as bf16; accumulation is f32.
  - DMA: the bulk stream (Wq -> kv slots -> Wo) runs on the Sync HWDGE ring in
    FIFO order with large contiguous-line transfers; small latency-sensitive
    transfers (gathers, collective staging, outputs) ride the Scalar ring.

The device program is specialized on seq_lens (static loop bounds); kernel()
re-builds/compiles whenever the derived schedule changes and memoizes it.
"""

import os
import numpy as np
import ml_dtypes

import concourse.bass as bass
import concourse.mybir as mybir
import concourse.tile as tile
from concourse import bacc
from concourse.bass_utils import run_bass_kernel_spmd

BF16 = ml_dtypes.bfloat16
P = 128
B, MAX_S, HID = 128, 4096, 5120
H, D, D_ROPE, D_V = 128, 128, 64, 128
NC, HPC, BPC = 8, 16, 16           # cores, heads/core, slots(batches)/core
QC = HPC * D                       # 2048 per-core q cols
KT = HID // P                      # 40 contraction tiles for Wq
WQ_G = 2                           # k-tiles per Wq DMA group
WO_PASS = 5                        # output passes of 1024 cols
SCALE = float(D) ** -0.5

dtb = mybir.dt.bfloat16
dtf = mybir.dt.float32

_PROGRAM_CACHE: dict = {}
LAST_RESULTS = None


def _install_ntff_hook():
    """The agent image's ``antenv`` lacks ``axon_hooks``; recreate it so
    run_bass_kernel_spmd(trace=True) can capture NTFF profiles via the
    libaxon ctypes path. Best-effort — failure just means no trace."""
    import sys
    import types
    try:
        import antenv.axon_hooks  # noqa: F401
        return
    except ImportError:
        pass
    try:
        from trn_agent_boot.trn_boot import _ntff_profile_via_ctypes
        hook = _ntff_profile_via_ctypes("/opt/axon/libaxon_pjrt.so")
        mod = types.ModuleType("antenv.axon_hooks")
        mod._hook = hook
        mod.set_axon_ntff_profile_hook = lambda h: setattr(mod, "_hook", h)
        mod.get_axon_ntff_profile_hook = lambda: mod._hook
        sys.modules["antenv.axon_hooks"] = mod
        import antenv
        antenv.axon_hooks = mod
    except Exception:
        pass


def _build_program(T, toff, NTT):
    """Build + compile the per-core bass program. T/toff: per-slot tile counts
    and tile offsets (static schedule, same on every core). NTT = sum(T)."""
    nc = bacc.Bacc("TRN2", target_bir_lowering=False, debug=False, num_devices=NC)

    ht_d = nc.dram_tensor("ht", [P, HID], dtb, kind="ExternalInput")
    wqt_d = nc.dram_tensor("wqt", [P, KT * QC], dtb, kind="ExternalInput")
    kv_d = nc.dram_tensor("kv", [P, NTT * 257], dtb, kind="ExternalInput")
    padc_d = nc.dram_tensor("padc", [P, BPC], dtf, kind="ExternalInput")
    wot_d = nc.dram_tensor("wot", [P, WO_PASS * HPC * 1024], dtb, kind="ExternalInput")
    idn_d = nc.dram_tensor("idn", [P, P], dtb, kind="ExternalInput")
    out_d = nc.dram_tensor("outp", [P, HID], dtf, kind="ExternalOutput")

    rg = [list(range(NC))]
    Exp = mybir.ActivationFunctionType.Exp
    Copy = mybir.ActivationFunctionType.Copy

    with tile.TileContext(nc) as tc:
        with (
            tc.tile_pool(name="cpool", bufs=1) as cpool,
            tc.tile_pool(name="wqpool", bufs=3) as wqpool,
            tc.tile_pool(name="kvpool", bufs=3) as kvpool,
            tc.tile_pool(name="etpool", bufs=3) as etpool,
            tc.tile_pool(name="spool", bufs=2) as spool,
            tc.tile_pool(name="qtpool", bufs=BPC) as qtpool,
            tc.tile_pool(name="lhpool", bufs=HPC) as lhpool,
            tc.tile_pool(name="wopool", bufs=2) as wopool,
            tc.tile_pool(name="opool", bufs=2) as opool,
            tc.tile_pool(name="dram", bufs=1, space="DRAM") as dram,
            tc.tile_pool(name="tps", bufs=2, space="PSUM") as tps,
        ):
            # ---- constants (scalar DMA ring: latency-sensitive small loads) ----
            idn = cpool.tile([P, P], dtb)
            nc.scalar.dma_start(idn[:], idn_d.ap())
            padc = cpool.tile([P, BPC], dtf)
            nc.scalar.dma_start(padc[:], padc_d.ap())

            # ---- bulk stream on the sync ring: ht, then Wq groups ----
            hts = cpool.tile([P, HID], dtb)
            nc.sync.dma_start(hts[:], ht_d.ap())

            qcc_in = dram.tile([P, QC], dtb)
            qoutA = dram.tile([P // 2, QC], dtb)
            qoutB = dram.tile([P // 2, QC], dtb)
            acc_in = dram.tile([P, BPC * D_V], dtb)
            acc_out = dram.tile([P, BPC * D_V], dtb)

            # ---- phase A: q projection (this core's 16 heads, all 128 batches)
            with tc.tile_pool(name="psA", bufs=1, space="PSUM") as psA:
                q_ps = psA.tile([P, QC], dtf)
                for g in range(KT // WQ_G):
                    wt = wqpool.tile([P, WQ_G * QC], dtb, tag="wq")
                    nc.sync.dma_start(wt[:], wqt_d.ap()[:, g * WQ_G * QC:(g + 1) * WQ_G * QC])
                    for k in range(WQ_G):
                        kk = g * WQ_G + k
                        for c0 in range(0, QC, 512):
                            nc.tensor.matmul(
                                q_ps[:, c0:c0 + 512],
                                lhsT=hts[:, kk * P:(kk + 1) * P],
                                rhs=wt[:, k * QC + c0:k * QC + c0 + 512],
                                start=(kk == 0),
                                stop=(kk == KT - 1),
                            )
                qc_sb = cpool.tile([P, QC], dtb)
                nc.vector.tensor_copy(qc_sb[:], q_ps[:])

            nc.scalar.dma_start(qcc_in[:], qc_sb[:])
            # reshard q: batch rows are laid out (half, core, slot%8) so each
            # 64-row half is a self-contained AllToAll
            nc.gpsimd.collective_compute(
                "AllToAll", mybir.AluOpType.bypass, replica_groups=rg,
                ins=[qcc_in.opt()[0:64]], outs=[qoutA.opt()],
            )
            nc.gpsimd.collective_compute(
                "AllToAll", mybir.AluOpType.bypass, replica_groups=rg,
                ins=[qcc_in.opt()[64:128]], outs=[qoutB.opt()],
            )

            # ---- phase B: attention over this core's 16 batches ----
            attn_cc = cpool.tile([P, BPC * D_V], dtb)

            def stage_qjT(j):
                """gather q for slot j from the resharded buffer + transpose"""
                qout = qoutA if j < 8 else qoutB
                j8 = j % 8
                qj = spool.tile([P, P], dtb, tag="qj")
                src = (
                    qout.opt().rearrange("(c j) f -> c j f", j=8)[:, j8:j8 + 1, :]
                    .squeeze(1)
                    .rearrange("c (hl d) -> c hl d", d=P)
                )
                nc.scalar.dma_start(qj[:], src)
                qjT_ps = tps.tile([P, P], dtb, tag="tps")
                nc.tensor.transpose(qjT_ps[:], qj[:], idn[:])
                qjT = qtpool.tile([P, P], dtb, tag="qjT")
                nc.vector.tensor_copy(qjT[:], qjT_ps[:])
                return qjT

            with (
                tc.tile_pool(name="scps", bufs=2, space="PSUM") as scps,
                tc.tile_pool(name="aps", bufs=2, space="PSUM") as aps,
            ):
                qjTs = {}
                for j in range(8):
                    qjTs[j] = stage_qjT(j)

                for j in range(BPC):
                    Tj = int(T[j])
                    base = 257 * int(toff[j])
                    qjT = qjTs[j]
                    kvt = kvpool.tile([P, Tj * 257], dtb, tag="kv")
                    nc.sync.dma_start(kvt[:], kv_d.ap()[:, base:base + Tj * 257])
                    vbase = Tj * P

                    groups = []
                    t0 = 0
                    while t0 < Tj:
                        groups.append((t0, min(4, Tj - t0)))
                        t0 += 4
                    G = len(groups)

                    attn_ps = aps.tile([P, D_V + 1], dtf, tag="attn")
                    # software pipeline: score MMs for group g+1 are issued on
                    # the PE queue before attention MMs of group g, so the PE
                    # never waits for the exp on the scalar engine.
                    sc_live = {}
                    et_live = {}

                    def score_group(g):
                        t0, nt = groups[g]
                        sc = scps.tile([P, nt * P], dtf, tag="sc")
                        for ti in range(nt):
                            nc.tensor.matmul(
                                sc[:, ti * P:(ti + 1) * P],
                                lhsT=kvt[:, (t0 + ti) * P:(t0 + ti + 1) * P],
                                rhs=qjT[:],
                                start=True, stop=True,
                            )
                        et = etpool.tile([P, nt * P], dtb, tag="et")
                        nc.scalar.activation(et[:], sc[:], Exp, bias=0.0, scale=SCALE)
                        sc_live[g] = sc
                        et_live[g] = et

                    def attn_group(g):
                        t0, nt = groups[g]
                        et = et_live.pop(g)
                        for ti in range(nt):
                            t = t0 + ti
                            nc.tensor.matmul(
                                attn_ps[:],
                                lhsT=et[:, ti * P:(ti + 1) * P],
                                rhs=kvt[:, vbase + t * 129:vbase + (t + 1) * 129],
                                start=(t == 0), stop=(t == Tj - 1),
                                skip_group_check=True,
                            )

                    score_group(0)
                    for g in range(1, G):
                        score_group(g)
                        attn_group(g - 1)
                    attn_group(G - 1)

                    # prefetch next half's q transposes while attention runs
                    if j == 7:
                        for jn in range(8, BPC):
                            qjTs[jn] = stage_qjT(jn)

                    ssum = spool.tile([P, 1], dtf, tag="ssum")
                    nc.vector.tensor_sub(ssum[:], attn_ps[:, D_V:D_V + 1], padc[:, j:j + 1])
                    recip = spool.tile([P, 1], dtf, tag="recip")
                    nc.vector.reciprocal(recip[:], ssum[:])
                    nc.scalar.activation(
                        attn_cc[:, j * D_V:(j + 1) * D_V], attn_ps[:, 0:D_V], Copy,
                        bias=0.0, scale=recip[:],
                    )

            nc.scalar.dma_start(acc_in[:], attn_cc[:])
            nc.gpsimd.collective_compute(
                "AllToAll", mybir.AluOpType.bypass, replica_groups=rg,
                ins=[acc_in.opt()], outs=[acc_out.opt()],
            )

            # ---- phase C: Wo partial (this core's 16 heads, all 128 batches) ----
            with tc.tile_pool(name="psC", bufs=2, space="PSUM") as psC:
                lhs_tiles = []
                for hl in range(HPC):
                    bm = spool.tile([P, P], dtb, tag="bm")
                    src = (
                        acc_out.opt().rearrange("(c hl) f -> c hl f", hl=HPC)[:, hl:hl + 1, :]
                        .squeeze(1)
                        .rearrange("c (j d) -> c j d", d=P)
                    )
                    nc.scalar.dma_start(bm[:], src)
                    lh_ps = tps.tile([P, P], dtb, tag="tps")
                    nc.tensor.transpose(lh_ps[:], bm[:], idn[:])
                    lh = lhpool.tile([P, P], dtb, tag="lh")
                    nc.vector.tensor_copy(lh[:], lh_ps[:])
                    lhs_tiles.append(lh)

                for ps in range(WO_PASS):
                    wot = wopool.tile([P, HPC * 1024], dtb, tag="wo")
                    nc.sync.dma_start(
                        wot[:], wot_d.ap()[:, ps * HPC * 1024:(ps + 1) * HPC * 1024]
                    )
                    wo_ps = psC.tile([P, 1024], dtf, tag="wops")
                    for kt in range(HPC):
                        for hf in range(2):
                            nc.tensor.matmul(
                                wo_ps[:, hf * 512:(hf + 1) * 512],
                                lhsT=lhs_tiles[kt][:],
                                rhs=wot[:, kt * 1024 + hf * 512:kt * 1024 + (hf + 1) * 512],
                                start=(kt == 0), stop=(kt == HPC - 1),
                            )
                    out_sb = opool.tile([P, 1024], dtf, tag="osb")
                    nc.vector.tensor_copy(out_sb[:], wo_ps[:])
                    nc.scalar.dma_start(out_d.ap()[:, ps * 1024:(ps + 1) * 1024], out_sb[:])

    nc.compile()
    return nc


def kernel(hidden_states, k_cache, v_cache, Wq, Wkv, Wo, positions, slot_mapping,
           seq_lens):
    global LAST_RESULTS
    h = np.asarray(hidden_states, np.float32)[:, -1, :]        # [B, HID]
    k_cache = np.asarray(k_cache, np.float32)
    v_cache = np.asarray(v_cache, np.float32)
    Wq = np.asarray(Wq, np.float32)
    Wkv = np.asarray(Wkv, np.float32)
    Wo = np.asarray(Wo, np.float32)
    seq = np.asarray(seq_lens).astype(np.int64)

    # ---- schedule: sort batches by length, slot j holds ranks [8j, 8j+8) ----
    order = np.argsort(-seq, kind="stable")
    batch_of = np.empty((NC, BPC), np.int64)
    for j in range(BPC):
        for c in range(NC):
            batch_of[c, j] = order[NC * j + c]
    # device row layout: r = (j//8)*64 + c*8 + (j%8)  (halves are contiguous)
    row_of = np.empty((NC, BPC), np.int64)
    for c in range(NC):
        for j in range(BPC):
            row_of[c, j] = (j // 8) * 64 + c * 8 + (j % 8)
    perm_rows = np.empty(B, np.int64)                          # device row -> batch
    for c in range(NC):
        for j in range(BPC):
            perm_rows[row_of[c, j]] = batch_of[c, j]

    L = np.array([int(seq[order[NC * j]]) for j in range(BPC)])
    T = (L + P - 1) // P
    toff = np.concatenate([[0], np.cumsum(T)])[:-1].astype(np.int64)
    NTT = int(T.sum())

    key = (NTT, tuple(int(t) for t in T))
    if key not in _PROGRAM_CACHE:
        _PROGRAM_CACHE.clear()
        _PROGRAM_CACHE[key] = _build_program(T, toff, NTT)
    nc = _PROGRAM_CACHE[key]

    # ---- host staging ----
    # new-token k/v (tiny projection, done in f32 exactly like the reference)
    kvnew = h @ Wkv                                            # [B, 576]
    k_new = kvnew[:, :D]
    v_new = kvnew[:, D:D + D_V]

    # h^T in device layout: [p, k*128 + r] = h_row(r)[k*128+p]
    h_sigma = h[perm_rows].astype(BF16)                        # [128, HID]
    ht_stage = np.ascontiguousarray(
        h_sigma.reshape(P, HID // P, P).transpose(2, 1, 0).reshape(P, HID)
    )
    idn_np = np.eye(P, dtype=BF16)

    in_maps = []
    for c in range(NC):
        # Wq columns for this core's 16 heads, transposed-tiled:
        # wqt[p, k*QC + col] = Wq[k*128+p, col_global]
        cols = [Wq[:, (HPC * c + hl) * (D + D_ROPE):(HPC * c + hl) * (D + D_ROPE) + D]
                for hl in range(HPC)]
        wq_shard = np.concatenate(cols, axis=1).astype(BF16)   # [5120, 2048]
        wqt_stage = np.ascontiguousarray(
            wq_shard.reshape(KT, P, QC).transpose(1, 0, 2).reshape(P, KT * QC)
        )
        # Wo rows for this core's heads, pass-major layout:
        # wot[p, ps*16*1024 + kt*1024 + cc] = Wo_shard[kt*128+p, ps*1024+cc]
        wo_shard = Wo[HPC * c * D_V:(HPC * c + HPC) * D_V, :].astype(BF16)  # [2048, 5120]
        wot_stage = np.ascontiguousarray(
            wo_shard.reshape(HPC, P, WO_PASS, 1024).transpose(1, 2, 0, 3)
            .reshape(P, WO_PASS * HPC * 1024)
        )

        kv_stage = np.zeros((P, NTT * 257), BF16)
        padc_stage = np.zeros((P, BPC), np.float32)
        for j in range(BPC):
            b = int(batch_of[c, j])
            S = int(seq[b])
            Tj = int(T[j])
            base = 257 * int(toff[j])
            # kT block: position 0 = new token, 1..S-1 = cache rows 0..S-2
            ktb = np.zeros((P, Tj * P), BF16)
            ktb[:, 0] = k_new[b].astype(BF16)
            ktb[:, 1:S] = k_cache[b, :S - 1, :].T.astype(BF16)
            kv_stage[:, base:base + Tj * P] = ktb
            # v block with ones column (softmax denominator via matmul)
            vblk = np.zeros((Tj * P, D_V + 1), BF16)
            vblk[0, :D_V] = v_new[b].astype(BF16)
            vblk[1:S, :D_V] = v_cache[b, :S - 1, :].astype(BF16)
            vblk[:, D_V] = 1.0
            kv_stage[:, base + Tj * P:base + Tj * 257] = (
                vblk.reshape(Tj, P, D_V + 1).transpose(1, 0, 2).reshape(P, Tj * 129)
            )
            padc_stage[:, j] = float(Tj * P - S)
        in_maps.append({
            "ht": ht_stage, "wqt": wqt_stage, "kv": kv_stage,
            "padc": padc_stage, "wot": wot_stage, "idn": idn_np,
        })

    trace = os.environ.get("BASS_KERNEL_TRACE", "0") == "1"
    if trace:
        _install_ntff_hook()
    res = run_bass_kernel_spmd(nc, in_maps, core_ids=list(range(NC)), trace=trace)
    LAST_RESULTS = res

    # out rows are in (c, j) order: row = c*16 + j  <->  batch_of[c, j]
    out_sigma = np.zeros((P, HID), np.float32)
    for c in range(NC):
        out_sigma += res.results[c]["outp"]
    out_full = np.empty((B, HID), np.float32)
    out_full[batch_of.reshape(-1)] = out_sigma
    return out_full.reshape(B, 1, HID)
